# revision 12
# baseline (speedup 1.0000x reference)
"""MoE layer (B=4, T=2048, C=1024, F=4096, E=8, top-2) on 8 trn2 NeuronCores.

Strategy: expert parallelism. The gate (tiny: N*C*E MACs) plus top-2
routing runs on the host as part of input sharding; each NeuronCore owns
one expert and runs the dense two-layer FFN over the tokens routed to it
(gathered + transposed + zero-padded to a common capacity so the SPMD
program is shape-uniform across cores). The host applies the combine
weights and scatter-adds per-expert outputs into the full output.

Device layout (everything "transposed" so weights are the stationary
matmul operand and tokens stream as the moving operand):
  xt  [C/128, 128, cap]  bf16   x gathered for this expert, transposed
  w1  [C/128, 128, F]    bf16   w1[e]  (partition dim = C slice)
  w2  [F/128, 128, C]    bf16   w2[e]  (partition dim = F slice)
  b1  [128, F/128]       f32    b1[e] transposed (partition = F%128)
  b2  [128, C/128]       f32    b2[e] transposed
  yt  [C/128, 128, cap]  f32    (h @ w2 + b2) transposed, combine weight
                                applied on host.
"""

import math

import numpy as np
import ml_dtypes

B, T, C, F, E, TOPK = 4, 2048, 1024, 4096, 8, 2
N_CORES = 8
KC = C // 128  # 8   C-slices (layer-1 contraction / layer-2 output)
KF = F // 128  # 32  F-slices (layer-1 output / layer-2 contraction)
TOK_TILE = 512

_BF16 = ml_dtypes.bfloat16

_nc_cache: dict[int, object] = {}


def _token_tiles(cap: int):
    """Split cap into equal-ish tiles of at most TOK_TILE tokens.

    Equal sizes keep every matmul's streaming time above the LDWEIGHTS
    shadow (a small tail tile would be weight-load-bound on the PE)."""
    n = -(-cap // TOK_TILE)
    base, rem = divmod(cap, n)
    tiles, off = [], 0
    for i in range(n):
        t = base + (1 if i < rem else 0)
        tiles.append((off, t))
        off += t
    return tiles


def build_moe_nc(cap: int, act: str = "Gelu"):
    """Build + compile the per-core Bass program for token capacity `cap`."""
    import concourse.mybir as mybir
    import concourse.tile as tile
    from concourse import bacc

    dt = mybir.dt
    GELU = getattr(mybir.ActivationFunctionType, act)
    IDENT = mybir.ActivationFunctionType.Identity

    nc = bacc.Bacc("TRN2", target_bir_lowering=False, debug=False)

    xt_d = nc.dram_tensor("xt", [KC, 128, cap], dt.bfloat16, kind="ExternalInput")
    w1_d = nc.dram_tensor("w1", [KC, 128, F], dt.bfloat16, kind="ExternalInput")
    w2_d = nc.dram_tensor("w2", [KF, 128, C], dt.bfloat16, kind="ExternalInput")
    b1_d = nc.dram_tensor("b1", [128, KF], dt.float32, kind="ExternalInput")
    b2_d = nc.dram_tensor("b2", [128, KC], dt.float32, kind="ExternalInput")
    yt_d = nc.dram_tensor("yt", [KC, 128, cap], dt.float32, kind="ExternalOutput")

    with tile.TileContext(nc) as tc:
        with (
            tc.tile_pool(name="wpool", bufs=1) as wpool,
            tc.tile_pool(name="xpool", bufs=3) as xpool,
            tc.tile_pool(name="hpool", bufs=1) as hpool,
            tc.tile_pool(name="ypool", bufs=4) as ypool,
            tc.tile_pool(name="pp", bufs=6, space="PSUM") as pp,
        ):
            tiles = _token_tiles(cap)

            def load_xt(off, tsz):
                # one tile per C-slice so each matmul only waits on the
                # 128-partition slab it actually streams
                xt_s = []
                for kc in range(KC):
                    xk = xpool.tile([128, tsz], dt.bfloat16, tag=f"xt_{kc}")
                    nc.sync.dma_start(xk[:], xt_d[kc, :, off : off + tsz])
                    xt_s.append(xk)
                return xt_s

            # DMA priority order: first token tile, then w1 in mf-chunk
            # order (layer 1 consumes w1 column-chunk h only once it gets
            # to mf block 8h..8h+7, so chunk-major delivery lets the PE
            # start ~2MB in instead of waiting for the full 8MB), then w2
            # (not read until layer 2, ~55us in).
            W1H = 8  # w1 column chunks
            HW = KF // W1H  # mf blocks per chunk
            FH = F // W1H
            w1_s = [[None] * W1H for _ in range(KC)]

            def load_w1(kc, h):
                w = wpool.tile([128, FH], dt.bfloat16, tag=f"w1_{kc}_{h}")
                nc.sync.dma_start(w[:], w1_d[kc, :, h * FH : (h + 1) * FH])
                w1_s[kc][h] = w

            # the very first matmul needs only w1[0][h0] + xt slice 0
            load_w1(0, 0)
            xt0_s = load_xt(*tiles[0])
            for kc in range(1, KC):
                load_w1(kc, 0)
            b1_s = wpool.tile([128, KF], dt.float32, tag="b1")
            b2_s = wpool.tile([128, KC], dt.float32, tag="b2")
            nc.sync.dma_start(b1_s[:], b1_d[:])
            nc.sync.dma_start(b2_s[:], b2_d[:])
            # interleave the rest of w1 with w2 so both streams land
            # before layer 1 / layer 2 respectively consume them
            w2_s = [None] * KF

            def load_w2(kf):
                w = wpool.tile([128, C], dt.bfloat16, tag=f"w2_{kf}")
                nc.sync.dma_start(w[:], w2_d[kf, :, :])
                w2_s[kf] = w

            next_w2 = 0
            for h in range(1, W1H):
                for kc in range(KC):
                    load_w1(kc, h)
                for _ in range(KF // (W1H - 1)):
                    if next_w2 < KF:
                        load_w2(next_w2)
                        next_w2 += 1
            while next_w2 < KF:
                load_w2(next_w2)
                next_w2 += 1

            for ti, (off, tsz) in enumerate(tiles):
                xt_s = xt0_s if ti == 0 else load_xt(off, tsz)

                # layer 1: h^T[f_blk, tok] = gelu(w1^T @ x^T + b1)
                ht_s = hpool.tile([128, KF, tsz], dt.bfloat16, tag="ht")
                for mf in range(KF):
                    h, mfh = divmod(mf, HW)
                    ps = pp.tile([128, tsz], dt.float32, tag="ps")
                    for kc in range(KC):
                        nc.tensor.matmul(
                            ps[:],
                            w1_s[kc][h][:, mfh * 128 : (mfh + 1) * 128],
                            xt_s[kc][:],
                            start=(kc == 0),
                            stop=(kc == KC - 1),
                        )
                    nc.scalar.activation(
                        ht_s[:, mf, :], ps[:], GELU, bias=b1_s[:, mf : mf + 1]
                    )

                # layer 2: y^T[c_blk, tok] = w2^T @ h^T + b2
                for mc in range(KC):
                    ps2 = pp.tile([128, tsz], dt.float32, tag="ps")
                    for kf in range(KF):
                        nc.tensor.matmul(
                            ps2[:],
                            w2_s[kf][:, mc * 128 : (mc + 1) * 128],
                            ht_s[:, kf, :],
                            start=(kf == 0),
                            stop=(kf == KF - 1),
                        )
                    y_s = ypool.tile([128, tsz], dt.float32, tag="y")
                    nc.scalar.activation(
                        y_s[:], ps2[:], IDENT, bias=b2_s[:, mc : mc + 1]
                    )
                    nc.sync.dma_start(yt_d[mc, :, off : off + tsz], y_s[:])

    nc.compile()
    return nc


def _route(x_flat, gate_w, gate_b):
    """Replicates reference gating: softmax -> top-2 -> renormalize."""
    logits = x_flat @ gate_w + gate_b  # [N, E] f32
    m = logits.max(-1, keepdims=True)
    p = np.exp(logits - m)
    p /= p.sum(-1, keepdims=True)
    # jax.lax.top_k: descending, ties -> lower index. Stable argsort matches.
    order = np.argsort(-p, axis=1, kind="stable")[:, :TOPK]  # [N, 2]
    top = np.take_along_axis(p, order, axis=1)
    wts = top / top.sum(-1, keepdims=True)
    return order, wts.astype(np.float32)


def run_moe(inputs: dict, trace: bool = False):
    """Returns (full_output [B,T,C] f32, BassKernelResults)."""
    from concourse.bass_utils import run_bass_kernel_spmd

    x = np.asarray(inputs["x"], dtype=np.float32)
    gate_w = np.asarray(inputs["gate_w"], dtype=np.float32)
    gate_b = np.asarray(inputs["gate_b"], dtype=np.float32)
    w1 = np.asarray(inputs["w1"], dtype=np.float32)
    b1 = np.asarray(inputs["b1"], dtype=np.float32)
    w2 = np.asarray(inputs["w2"], dtype=np.float32)
    b2 = np.asarray(inputs["b2"], dtype=np.float32)

    xf = x.reshape(-1, C)
    order, wts = _route(xf, gate_w, gate_b)

    idx = []
    comb = []
    for e in range(E):
        mask = order == e  # [N, 2]
        rows = np.nonzero(mask.any(axis=1))[0]
        idx.append(rows)
        comb.append((wts[rows] * mask[rows]).sum(axis=1).astype(np.float32))
    max_n = max(len(r) for r in idx)
    cap = max(64, max_n)

    if cap not in _nc_cache:
        _nc_cache[cap] = build_moe_nc(cap)
    nc = _nc_cache[cap]

    in_maps = []
    for e in range(E):
        xt = np.zeros((C, cap), dtype=_BF16)
        xt[:, : len(idx[e])] = xf[idx[e]].T
        in_maps.append(
            {
                "xt": xt.reshape(KC, 128, cap),
                "w1": np.ascontiguousarray(w1[e].astype(_BF16)).reshape(KC, 128, F),
                "w2": np.ascontiguousarray(w2[e].astype(_BF16)).reshape(KF, 128, C),
                "b1": np.ascontiguousarray(b1[e].reshape(KF, 128).T),
                "b2": np.ascontiguousarray(b2[e].reshape(KC, 128).T),
            }
        )

    res = run_bass_kernel_spmd(nc, in_maps, list(range(N_CORES)), trace=trace)

    out = np.zeros_like(xf)
    for e in range(E):
        n_e = len(idx[e])
        if n_e == 0:
            continue
        y = res.results[e]["yt"].reshape(C, cap)[:, :n_e].T  # [n_e, C]
        out[idx[e]] += comb[e][:, None] * y
    return out.reshape(B, T, C), res


def kernel(x, gate_w, gate_b, w1, b1, w2, b2):
    out, _ = run_moe(
        {
            "x": x,
            "gate_w": gate_w,
            "gate_b": gate_b,
            "w1": w1,
            "b1": b1,
            "w2": w2,
            "b2": b2,
        }
    )
    return out


# revision 16
# speedup vs baseline: 1.0271x; 1.0271x over previous
"""MoE layer (B=4, T=2048, C=1024, F=4096, E=8, top-2) on 8 trn2 NeuronCores.

Strategy: expert parallelism. The gate (tiny: N*C*E MACs) plus top-2
routing runs on the host as part of input sharding; each NeuronCore owns
one expert and runs the dense two-layer FFN over the tokens routed to it
(gathered + transposed + zero-padded to a common capacity so the SPMD
program is shape-uniform across cores). The host applies the combine
weights and scatter-adds per-expert outputs into the full output.

Device layout (everything "transposed" so weights are the stationary
matmul operand and tokens stream as the moving operand):
  xt  [C/128, 128, cap]  bf16   x gathered for this expert, transposed
  w1  [C/128, 128, F]    bf16   w1[e]  (partition dim = C slice)
  w2  [F/128, 128, C]    bf16   w2[e]  (partition dim = F slice)
  b1  [128, F/128]       f32    b1[e] transposed (partition = F%128)
  b2  [128, C/128]       f32    b2[e] transposed
  yt  [C/128, 128, cap]  f32    (h @ w2 + b2) transposed, combine weight
                                applied on host.
"""

import math

import numpy as np
import ml_dtypes

B, T, C, F, E, TOPK = 4, 2048, 1024, 4096, 8, 2
N_CORES = 8
KC = C // 128  # 8   C-slices (layer-1 contraction / layer-2 output)
KF = F // 128  # 32  F-slices (layer-1 output / layer-2 contraction)
TOK_TILE = 512

_BF16 = ml_dtypes.bfloat16

_nc_cache: dict[int, object] = {}


def _token_tiles(cap: int):
    """Split cap into equal-ish tiles of at most TOK_TILE tokens.

    Equal sizes keep every matmul's streaming time above the LDWEIGHTS
    shadow (a small tail tile would be weight-load-bound on the PE)."""
    n = -(-cap // TOK_TILE)
    base, rem = divmod(cap, n)
    tiles, off = [], 0
    for i in range(n):
        t = base + (1 if i < rem else 0)
        tiles.append((off, t))
        off += t
    return tiles


def build_moe_nc(cap: int, act: str = "Gelu"):
    """Build + compile the per-core Bass program for token capacity `cap`."""
    import concourse.mybir as mybir
    import concourse.tile as tile
    from concourse import bacc

    dt = mybir.dt
    GELU = getattr(mybir.ActivationFunctionType, act)
    IDENT = mybir.ActivationFunctionType.Identity

    nc = bacc.Bacc("TRN2", target_bir_lowering=False, debug=False)

    xt_d = nc.dram_tensor("xt", [KC, 128, cap], dt.bfloat16, kind="ExternalInput")
    w1_d = nc.dram_tensor("w1", [KC, 128, F], dt.bfloat16, kind="ExternalInput")
    w2_d = nc.dram_tensor("w2", [KF, 128, C], dt.bfloat16, kind="ExternalInput")
    b1_d = nc.dram_tensor("b1", [128, KF], dt.float32, kind="ExternalInput")
    b2_d = nc.dram_tensor("b2", [128, KC], dt.float32, kind="ExternalInput")
    yt_d = nc.dram_tensor("yt", [KC, 128, cap], dt.float32, kind="ExternalOutput")

    with tile.TileContext(nc) as tc:
        with (
            tc.tile_pool(name="wpool", bufs=1) as wpool,
            tc.tile_pool(name="xpool", bufs=3) as xpool,
            tc.tile_pool(name="hpool", bufs=1) as hpool,
            tc.tile_pool(name="ypool", bufs=4) as ypool,
            tc.tile_pool(name="pp", bufs=8, space="PSUM") as pp,
        ):
            tiles = _token_tiles(cap)

            def load_xt(off, tsz):
                # one tile per C-slice so each matmul only waits on the
                # 128-partition slab it actually streams
                xt_s = []
                for kc in range(KC):
                    xk = xpool.tile([128, tsz], dt.bfloat16, tag=f"xt_{kc}")
                    nc.sync.dma_start(xk[:], xt_d[kc, :, off : off + tsz])
                    xt_s.append(xk)
                return xt_s

            # DMA priority order: first token tile, then w1 in mf-chunk
            # order (layer 1 consumes w1 column-chunk h only once it gets
            # to mf block 8h..8h+7, so chunk-major delivery lets the PE
            # start ~2MB in instead of waiting for the full 8MB), then w2
            # (not read until layer 2, ~55us in).
            W1H = 4  # w1 column chunks
            HW = KF // W1H  # mf blocks per chunk
            FH = F // W1H
            w1_s = [[None] * W1H for _ in range(KC)]

            def load_w1(kc, h):
                w = wpool.tile([128, FH], dt.bfloat16, tag=f"w1_{kc}_{h}")
                nc.sync.dma_start(w[:], w1_d[kc, :, h * FH : (h + 1) * FH])
                w1_s[kc][h] = w

            # the very first matmul needs only w1[0][h0] + xt slice 0
            load_w1(0, 0)
            xt0_s = load_xt(*tiles[0])
            for kc in range(1, KC):
                load_w1(kc, 0)
            b1_s = wpool.tile([128, KF], dt.float32, tag="b1")
            b2_s = wpool.tile([128, KC], dt.float32, tag="b2")
            nc.sync.dma_start(b1_s[:], b1_d[:])
            nc.sync.dma_start(b2_s[:], b2_d[:])
            for h in range(1, W1H):
                for kc in range(KC):
                    load_w1(kc, h)
            w2_s = []
            for kf in range(KF):
                w = wpool.tile([128, C], dt.bfloat16, tag=f"w2_{kf}")
                nc.sync.dma_start(w[:], w2_d[kf, :, :])
                w2_s.append(w)

            for ti, (off, tsz) in enumerate(tiles):
                xt_s = xt0_s if ti == 0 else load_xt(off, tsz)

                # layer 1: h^T[f_blk, tok] = gelu(w1^T @ x^T + b1)
                ht_s = hpool.tile([128, KF, tsz], dt.bfloat16, tag="ht")
                mf_start = 0
                if ti == 0:
                    # kc-outer warmup over the first HW mf groups: the PE
                    # can start on w1[0][h0] alone (256KB) and consume the
                    # weight DMA stream chunk-by-chunk instead of stalling
                    # for the whole 2MB h0 block within the first ~2us.
                    mf_start = HW
                    ps_w = [
                        pp.tile([128, tsz], dt.float32, tag="ps", name=f"ps_w{i}")
                        for i in range(HW)
                    ]
                    for kc in range(KC):
                        for mfh in range(HW):
                            nc.tensor.matmul(
                                ps_w[mfh][:],
                                w1_s[kc][0][:, mfh * 128 : (mfh + 1) * 128],
                                xt_s[kc][:],
                                start=(kc == 0),
                                stop=(kc == KC - 1),
                            )
                    for mfh in range(HW):
                        nc.scalar.activation(
                            ht_s[:, mfh, :], ps_w[mfh][:], GELU,
                            bias=b1_s[:, mfh : mfh + 1],
                        )
                for mf in range(mf_start, KF):
                    h, mfh = divmod(mf, HW)
                    ps = pp.tile([128, tsz], dt.float32, tag="ps")
                    for kc in range(KC):
                        nc.tensor.matmul(
                            ps[:],
                            w1_s[kc][h][:, mfh * 128 : (mfh + 1) * 128],
                            xt_s[kc][:],
                            start=(kc == 0),
                            stop=(kc == KC - 1),
                        )
                    nc.scalar.activation(
                        ht_s[:, mf, :], ps[:], GELU, bias=b1_s[:, mf : mf + 1]
                    )

                # layer 2: y^T[c_blk, tok] = w2^T @ h^T + b2
                for mc in range(KC):
                    ps2 = pp.tile([128, tsz], dt.float32, tag="ps")
                    for kf in range(KF):
                        nc.tensor.matmul(
                            ps2[:],
                            w2_s[kf][:, mc * 128 : (mc + 1) * 128],
                            ht_s[:, kf, :],
                            start=(kf == 0),
                            stop=(kf == KF - 1),
                        )
                    y_s = ypool.tile([128, tsz], dt.float32, tag="y")
                    nc.scalar.activation(
                        y_s[:], ps2[:], IDENT, bias=b2_s[:, mc : mc + 1]
                    )
                    nc.sync.dma_start(yt_d[mc, :, off : off + tsz], y_s[:])

    nc.compile()
    return nc


def _route(x_flat, gate_w, gate_b):
    """Replicates reference gating: softmax -> top-2 -> renormalize."""
    logits = x_flat @ gate_w + gate_b  # [N, E] f32
    m = logits.max(-1, keepdims=True)
    p = np.exp(logits - m)
    p /= p.sum(-1, keepdims=True)
    # jax.lax.top_k: descending, ties -> lower index. Stable argsort matches.
    order = np.argsort(-p, axis=1, kind="stable")[:, :TOPK]  # [N, 2]
    top = np.take_along_axis(p, order, axis=1)
    wts = top / top.sum(-1, keepdims=True)
    return order, wts.astype(np.float32)


def run_moe(inputs: dict, trace: bool = False):
    """Returns (full_output [B,T,C] f32, BassKernelResults)."""
    from concourse.bass_utils import run_bass_kernel_spmd

    x = np.asarray(inputs["x"], dtype=np.float32)
    gate_w = np.asarray(inputs["gate_w"], dtype=np.float32)
    gate_b = np.asarray(inputs["gate_b"], dtype=np.float32)
    w1 = np.asarray(inputs["w1"], dtype=np.float32)
    b1 = np.asarray(inputs["b1"], dtype=np.float32)
    w2 = np.asarray(inputs["w2"], dtype=np.float32)
    b2 = np.asarray(inputs["b2"], dtype=np.float32)

    xf = x.reshape(-1, C)
    order, wts = _route(xf, gate_w, gate_b)

    idx = []
    comb = []
    for e in range(E):
        mask = order == e  # [N, 2]
        rows = np.nonzero(mask.any(axis=1))[0]
        idx.append(rows)
        comb.append((wts[rows] * mask[rows]).sum(axis=1).astype(np.float32))
    max_n = max(len(r) for r in idx)
    cap = max(64, max_n)

    if cap not in _nc_cache:
        _nc_cache[cap] = build_moe_nc(cap)
    nc = _nc_cache[cap]

    in_maps = []
    for e in range(E):
        xt = np.zeros((C, cap), dtype=_BF16)
        xt[:, : len(idx[e])] = xf[idx[e]].T
        in_maps.append(
            {
                "xt": xt.reshape(KC, 128, cap),
                "w1": np.ascontiguousarray(w1[e].astype(_BF16)).reshape(KC, 128, F),
                "w2": np.ascontiguousarray(w2[e].astype(_BF16)).reshape(KF, 128, C),
                "b1": np.ascontiguousarray(b1[e].reshape(KF, 128).T),
                "b2": np.ascontiguousarray(b2[e].reshape(KC, 128).T),
            }
        )

    res = run_bass_kernel_spmd(nc, in_maps, list(range(N_CORES)), trace=trace)

    out = np.zeros_like(xf)
    for e in range(E):
        n_e = len(idx[e])
        if n_e == 0:
            continue
        y = res.results[e]["yt"].reshape(C, cap)[:, :n_e].T  # [n_e, C]
        out[idx[e]] += comb[e][:, None] * y
    return out.reshape(B, T, C), res


def kernel(x, gate_w, gate_b, w1, b1, w2, b2):
    out, _ = run_moe(
        {
            "x": x,
            "gate_w": gate_w,
            "gate_b": gate_b,
            "w1": w1,
            "b1": b1,
            "w2": w2,
            "b2": b2,
        }
    )
    return out
